# revision 65
# baseline (speedup 1.0000x reference)
"""Two-layer LSTM (H=51) over [B=4096, T=256] on 8 NeuronCores.

Data-parallel over batch (512 per core), skewed software pipeline over T+2
phases: phase q computes layer-1 of step q together with layer-2 of step
q-1 (merged into the same matmuls), and the linear head of step q-2.

Per phase (per batch-group of 256):
  - 4 merged gate matmuls: one lhsT carries BOTH layers' weights
    (layer-1 -> output rows 0..50, layer-2 -> rows 64..114), two gate
    banks per PSUM tile (assignment configurable).
  - x is prefetched 4 steps per DMA into 12 rotating rhs partition slots
    (rows 116..127); each of the 12 lhsT variants reads its own x row.
  - Cell math (h stored undoubled, c stored doubled ct=2c):
    i uses the half-angle trick ti=tanh(zi/2) (0.5 in weights), g is
    tanh, f and o are real Sigmoids (one act table serves both funcs);
    v=(ti+1)*tg (=2*sig_i*gt), u=sf*ct, ct'=u+v, tcl=tanh(ct'/2),
    ht=so*tcl.  tanh/sigmoid on ACT, v/u/ct'/ht on DVE, head staging
    on DVE, all gate activations written to SBUF tiles.

Hardware legality notes (bir verifier):
  - Pool/gpsimd cannot access PSUM at all and has no scalar_tensor_tensor
    (and its tensor ops are priced at Q7-software rates in the cost
    model: launch 95ns + /0.42-0.6 efficiency), which is why everything
    elementwise ended up on DVE.
  - Other engines may read at most ONE non-scalar input from PSUM
    (hence gate activations go PSUM -> SBUF, not in-place).
"""

import numpy as np

H = 51
T_FULL = 256
B_FULL = 4096
N_CORES = 8

ROW_H1 = 0      # rows 0..50: ht1 (= 2*h1)
ROW_H2 = 64     # rows 64..114: ht2 (= 2*h2)  (64: 32-aligned lhsT base for head)
ROW_ONES = 115  # bias row
ROW_X0 = 116    # rows 116..127: 12 x slots (3 rotating DMA banks of 4)
NXSLOT = 12
XBANK = 4       # steps per x-prefetch DMA
K_STK = 128
GP = 115        # gate-row space: 0..50 layer1, 51..63 junk, 64..114 layer2
MW = 115        # matmul output width per bank

N_VAR = NXSLOT * 4          # merged variant banks
L1B = N_VAR                 # 4 layer-1-only banks (phase 0, x slot 0)
L2B = N_VAR + 4             # 4 layer-2-only banks (phase T)
HEADCOL = (N_VAR + 8) * MW  # head column
NCOL = HEADCOL + 1

# schedule/assignment knobs (sweepable):
#   t1/t2: gate pair per PSUM tile; t?_out: tanh dest ("sb" SBUF tile,
#   "ip" in-place PSUM); t?_split: one tanh instr per gate instead of a
#   joint one; engines: "dve" | "pool" (pool requires SBUF operands).
CONFIG = {
    "t1": "ig", "t2": "fo", "t1_out": "sb", "t2_out": "sb",
    "t1_split": False, "t2_split": True,
    "v_eng": "dve", "u_eng": "dve", "ct_eng": "dve", "ht_eng": "dve",
    # f_sig: f-gate is a real Sigmoid (joint [f,o] sigmoid instr);
    # u = sf*ct and ct' = u+v become plain tensor_tensor ops.
    "f_sig": True,
    # ht_eng "split": ht computed in two column-halves in parallel,
    # cols [0:ht_dve_cols] on DVE and the rest on Pool.
    "ht_dve_cols": 176,
    # issue tcl/ht inside each group's block instead of a shared tail loop
    "tails_inline": True,
    # defer the ht ops of both groups to the end of the phase (tcl stays
    # inline) so a waiting ht doesn't head-of-line-block the next group's
    # DVE work
    "ht_defer": True,
    # write the f sigmoid in-place to PSUM (u then reads its single
    # allowed PSUM input); requires t2_split
    "f_out_ip": False,
    # bf16 elementwise chain: gate activations, u/v/ct tiles in bf16 so
    # the TensorTensor ops (u, ct) hit the DVE 2x_1p mode (327 -> 193ns).
    # Matmuls, PSUM, stk state stay f32r.
    "ew_bf16": True,
    # bf16 state + weights + matmuls: stk/wg/xT in bf16 (1cyc/row at any
    # width, no f32r N=1 restriction for the head, ht gets 2x too).
    "mm_bf16": True,
    # emission-order perturbations: alternate group order by phase
    # parity; reverse the deferred-tail group order
    "pingpong": True,
    "tails_rev": False,
    # parity-alternate the activation-tile iteration and matmul bank order
    "act_pp": False,
    "mm_pp": True,
    # seeded per-phase emission-order randomization (overrides the
    # parity knobs when not None): hash(q, seed) picks group/bank/tile/
    # tail order per phase. Pure scheduling perturbation, math unchanged.
    "sched_seed": None,
    # explicit odd-phase matmul bank permutation (overrides mm_pp)
    "mm_odd_order": None,
    # allocate tile2's activation-output tile in PSUM (separate tile, NOT
    # in-place): ACT bubble 185->143 for f/o; u and ht then each read
    # their single allowed PSUM input on DVE
    "t2_sb_psum": False,
}


def _gate_order():
    return list(CONFIG["t1"]) + list(CONFIG["t2"])


def _build_weights(W_ih1, W_hh1, b_ih1, b_hh1, W_ih2, W_hh2, b_ih2, b_hh2,
                   W_lin, b_lin):
    """Host-side packing of lhsT weight banks -> WG [K_STK, NCOL] f32."""
    b1 = (b_ih1 + b_hh1).astype(np.float64)
    b2 = (b_ih2 + b_hh2).astype(np.float64)
    idx = {"i": np.arange(0, H), "f": np.arange(H, 2 * H),
           "g": np.arange(2 * H, 3 * H), "o": np.arange(3 * H, 4 * H)}
    WG = np.zeros((K_STK, NCOL), dtype=np.float64)
    order = _gate_order()

    f_sig = CONFIG["f_sig"]

    def fill_bank(col0, gate, l1, l2, xslot):
        # h is stored UNdoubled; c doubled (ct=2c). i (and f unless
        # f_sig) use the tanh-half-angle trick (s=0.5); g is tanh;
        # o (and f if f_sig) are real Sigmoids.
        r = idx[gate]
        s = 0.5 if (gate == "i" or (gate == "f" and not f_sig)) else 1.0
        if l1:
            c1 = slice(col0, col0 + H)  # output rows 0..50
            WG[ROW_H1:ROW_H1 + H, c1] += s * W_hh1[r, :].T
            WG[ROW_ONES, c1] += s * b1[r]
            WG[ROW_X0 + xslot, c1] += s * W_ih1[r, 0]
        if l2:
            c2 = slice(col0 + ROW_H2, col0 + ROW_H2 + H)  # rows 64..114
            WG[ROW_H1:ROW_H1 + H, c2] += s * W_ih2[r, :].T
            WG[ROW_H2:ROW_H2 + H, c2] += s * W_hh2[r, :].T
            WG[ROW_ONES, c2] += s * b2[r]

    for slot in range(NXSLOT):
        for bi, gate in enumerate(order):
            fill_bank((slot * 4 + bi) * MW, gate, True, True, slot)
    for bi, gate in enumerate(order):
        fill_bank((L1B + bi) * MW, gate, True, False, 0)
        fill_bank((L2B + bi) * MW, gate, False, True, 0)
    WG[ROW_H2:ROW_H2 + H, HEADCOL] = W_lin[0, :]
    WG[ROW_ONES, HEADCOL] = float(np.asarray(b_lin).reshape(-1)[0])
    return np.ascontiguousarray(WG).astype(np.float32)


def build_core_kernel(T, B, groups=2, use_f32r=True):
    """Per-core Bass kernel. Inputs: xT [T+1, B] (row 0 = ones), WG.
    Output: out_bt [B, T]."""
    import concourse.bacc as bacc
    import concourse.mybir as mybir
    from concourse.tile import TileContext

    fp = mybir.dt.float32
    mm_bf16 = CONFIG["mm_bf16"]
    if mm_bf16:
        fpr = mybir.dt.bfloat16
    else:
        fpr = mybir.dt.float32r if use_f32r else fp
    few = mybir.dt.bfloat16 if CONFIG["ew_bf16"] else fp
    Bg = B // groups
    assert B % groups == 0 and Bg % 128 == 0

    nc = bacc.Bacc("TRN2", target_bir_lowering=False, debug=False)
    xT = nc.dram_tensor("xT", [T + 1, B], fpr, kind="ExternalInput")
    WG = nc.dram_tensor("WG", [K_STK, NCOL], fpr, kind="ExternalInput")
    out_bt = nc.dram_tensor("out_bt", [B, T], fp, kind="ExternalOutput")

    C = min(128, T)          # head columns buffered in PSUM between flushes
    assert T % C == 0
    nchunk = B // 128
    assert T % XBANK == 0

    add = mybir.AluOpType.add
    mult = mybir.AluOpType.mult
    tanh = mybir.ActivationFunctionType.Tanh
    sigmoid = mybir.ActivationFunctionType.Sigmoid
    f_sig = CONFIG["f_sig"]
    gfunc = {"i": tanh, "g": tanh,
             "f": sigmoid if f_sig else tanh, "o": sigmoid}
    eng = {"dve": nc.vector, "pool": nc.gpsimd}
    v_eng = eng[CONFIG["v_eng"]]
    u_eng = eng[CONFIG["u_eng"]]
    ct_eng = eng[CONFIG["ct_eng"]]
    tails_inline = CONFIG["tails_inline"]
    ht_split = CONFIG["ht_eng"] == "split"
    ht_eng = None if ht_split else eng[CONFIG["ht_eng"]]
    tiles_cfg = [(CONFIG["t1"], CONFIG["t1_out"], CONFIG["t1_split"]),
                 (CONFIG["t2"], CONFIG["t2_out"], CONFIG["t2_split"])]
    order = _gate_order()
    gate_tile = {}
    for tix, (gates, out, split) in enumerate(tiles_cfg):
        for ci, gname in enumerate(gates):
            gate_tile[gname] = (tix, ci)
    sb_gate = {g: tiles_cfg[gate_tile[g][0]][1] == "sb" for g in "igfo"}
    # legality: pool needs SBUF operands and supports only tensor_tensor
    # (no scalar_tensor_tensor); others: <=1 PSUM input per instruction
    assert sb_gate["i"] or sb_gate["g"], "v would read 2 PSUM inputs"
    assert CONFIG["v_eng"] != "pool", "v is stt; unsupported on Pool"
    if CONFIG["u_eng"] == "pool":
        assert f_sig and sb_gate["f"]
    if CONFIG["ct_eng"] == "pool":
        assert f_sig
    if CONFIG["ht_eng"] in ("pool", "split"):
        assert sb_gate["o"]

    with TileContext(nc) as tc:
        with (
            tc.tile_pool(name="persist", bufs=1) as persist,
            tc.tile_pool(name="gpsum", bufs=1, space="PSUM") as gpsum,
            tc.tile_pool(name="opsum", bufs=1, space="PSUM") as opsum,
        ):
            wg = persist.tile([K_STK, NCOL], fpr)
            # boundary banks + head column first (phase 0 needs them),
            # then the variant banks in parallel chunks
            c_bnd = L1B * MW
            nc.sync.dma_start(out=wg[:, c_bnd:NCOL], in_=WG[:, c_bnd:NCOL])
            nvar4 = (c_bnd // 4) // MW * MW
            for j in range(4):
                c0, c1 = j * nvar4, (j + 1) * nvar4 if j < 3 else c_bnd
                nc.sync.dma_start(out=wg[:, c0:c1], in_=WG[:, c0:c1])

            stk = persist.tile([K_STK, B], fpr, tag="stk")
            ctt = persist.tile([GP, B], few, tag="ctt")
            if mm_bf16:
                nc.vector.memset(stk[:, :], 0.0)
            else:
                nc.vector.memset(stk[:, :].bitcast(fp), 0.0)
            nc.vector.memset(ctt[:, :], 0.0)
            nc.sync.dma_start(out=stk[ROW_ONES:ROW_ONES + 1, :],
                              in_=xT[0:1, :])
            # preload x windows 0 and 4 (phases 0..7)
            nc.sync.dma_start(out=stk[ROW_X0:ROW_X0 + XBANK, :],
                              in_=xT[1:1 + XBANK, :])
            if T > XBANK:
                nc.sync.dma_start(
                    out=stk[ROW_X0 + XBANK:ROW_X0 + 2 * XBANK, :],
                    in_=xT[1 + XBANK:1 + 2 * XBANK, :])

            gpt = [[gpsum.tile([GP, 2 * Bg], fp, tag=f"gp{t}{g}",
                               name=f"gp{t}{g}") for t in range(2)]
                   for g in range(groups)]
            def _sb_pool(t):
                return gpsum if (t == 1 and CONFIG["t2_sb_psum"]) else persist

            sbt = [[_sb_pool(t).tile([GP, 2 * Bg], few, tag=f"sb{t}{g}",
                                     name=f"sb{t}{g}")
                    if tiles_cfg[t][1] == "sb" else None for t in range(2)]
                   for g in range(groups)]
            pos = opsum.tile([128, nchunk * C], fp, tag="pos")
            us = [persist.tile([GP, Bg], few, tag=f"u{g}", name=f"u{g}")
                  for g in range(groups)]
            vs = [persist.tile([GP, Bg], few, tag=f"v{g}", name=f"v{g}")
                  for g in range(groups)]
            tcls = [persist.tile([GP, Bg], few, tag=f"tcl{g}", name=f"tcl{g}")
                    for g in range(groups)]
            ost = persist.tile([128, nchunk * C], fp, tag="ost", name="ost")

            def gate_res(g, gname):
                tix, ci = gate_tile[gname]
                src = sbt[g][tix] if tiles_cfg[tix][1] == "sb" else gpt[g][tix]
                if gname == "f" and CONFIG["f_out_ip"]:
                    src = gpt[g][tix]
                return src[:, ci * Bg:(ci + 1) * Bg]

            def emit_tail(g, part="both"):
                cols = slice(g * Bg, (g + 1) * Bg)
                if part in ("both", "tcl"):
                    # tcl = tanh(ct/2) = tanh(c)
                    nc.scalar.activation(tcls[g], ctt[:, cols], tanh,
                                         scale=0.5)
                if part == "tcl":
                    return
                # ht = so * tcl  (h stored undoubled; so = sigmoid)
                so = gate_res(g, "o")
                if ht_split:
                    hd = CONFIG["ht_dve_cols"]
                    c0 = g * Bg
                    nc.vector.tensor_tensor(
                        stk[ROW_H1:ROW_H1 + GP, c0:c0 + hd],
                        so[:, 0:hd], tcls[g][:, 0:hd], mult)
                    nc.gpsimd.tensor_tensor(
                        stk[ROW_H1:ROW_H1 + GP, c0 + hd:c0 + Bg],
                        so[:, hd:Bg], tcls[g][:, hd:Bg], mult)
                else:
                    ht_eng.tensor_tensor(
                        stk[ROW_H1:ROW_H1 + GP, cols], so, tcls[g], mult)

            for q in range(T + 2):
                # ---- x prefetch: window w = q+6 (covers phases w..w+3),
                # rotating over 3 banks of 4 partition rows
                w = q + 6
                if w % XBANK == 0 and w < T:
                    bank = (w // XBANK) % (NXSLOT // XBANK)
                    nc.sync.dma_start(
                        out=stk[ROW_X0 + bank * XBANK:
                                ROW_X0 + (bank + 1) * XBANK, :],
                        in_=xT[1 + w:1 + w + XBANK, :])

                seed = CONFIG["sched_seed"]
                if seed is not None:
                    hbits = (q * 2654435761 + seed * 40503) >> 4
                else:
                    hbits = None
                gorder = list(range(groups))
                if hbits is not None:
                    if hbits & 1:
                        gorder = gorder[::-1]
                elif CONFIG["pingpong"] and q % 2 == 1:
                    gorder = gorder[::-1]
                for g in gorder:
                    cols = slice(g * Bg, (g + 1) * Bg)
                    if q > T:
                        continue
                    if q == 0:
                        vb = L1B
                    elif q == T:
                        vb = L2B
                    else:
                        vb = (q % NXSLOT) * 4
                    rhs = stk[0:K_STK, cols]
                    border = list(range(4))
                    if CONFIG["mm_odd_order"] is not None and q % 2 == 1:
                        border = list(CONFIG["mm_odd_order"])
                    elif hbits is not None:
                        if hbits & 2:
                            border = border[::-1]
                    elif CONFIG["mm_pp"] and q % 2 == 1:
                        border = border[::-1]
                    for bi in border:
                        tix, ci = gate_tile[order[bi]]
                        nc.tensor.matmul(
                            gpt[g][tix][0:GP, ci * Bg:(ci + 1) * Bg],
                            wg[0:K_STK, (vb + bi) * MW:(vb + bi + 1) * MW],
                            rhs, start=True, stop=True)
                    # tanh per tile (joint or split), v as soon as i,g are
                    # done, u as soon as f is done
                    done = set()
                    emitted_v = emitted_u = False
                    tlist = list(enumerate(tiles_cfg))
                    if hbits is not None:
                        if hbits & 4:
                            tlist = tlist[::-1]
                    elif CONFIG["act_pp"] and q % 2 == 1:
                        tlist = tlist[::-1]
                    for tix, (gates, out, split) in tlist:
                        src = gpt[g][tix]
                        dst = sbt[g][tix] if out == "sb" else src
                        if split:
                            for ci, gname in enumerate(gates):
                                gdst = dst
                                if gname == "f" and CONFIG["f_out_ip"]:
                                    gdst = src
                                nc.scalar.activation(
                                    gdst[:, ci * Bg:(ci + 1) * Bg],
                                    src[:, ci * Bg:(ci + 1) * Bg],
                                    gfunc[gname])
                                done.add(gname)
                                emitted_v, emitted_u = _maybe_uv(
                                    nc, g, cols, done, emitted_v, emitted_u,
                                    v_eng, u_eng, vs, us, ctt, gate_res,
                                    add, mult, f_sig)
                        else:
                            # one func per instruction
                            assert len({gfunc[x] for x in gates}) == 1
                            nc.scalar.activation(dst[:, :], src[:, :],
                                                 gfunc[gates[0]])
                            done.update(gates)
                            emitted_v, emitted_u = _maybe_uv(
                                nc, g, cols, done, emitted_v, emitted_u,
                                v_eng, u_eng, vs, us, ctt, gate_res,
                                add, mult, f_sig)
                    if f_sig:
                        # ct' = u + v  (u = sf*ct)
                        ct_eng.tensor_tensor(ctt[:, cols], us[g], vs[g], add)
                    else:
                        # ct' = 0.5*u + v  (u = (tf+1)*ct)
                        ct_eng.scalar_tensor_tensor(
                            ctt[:, cols], us[g], 0.5, vs[g], mult, add)
                    if tails_inline:
                        emit_tail(g, "tcl" if CONFIG["ht_defer"] else "both")

                # ---- head (reads stk h2; with inline tails this
                # phase's ht is already in stk, so the head step shifts)
                # heads read h2[t]: the phase offset depends on whether
                # this phase's ht has already been issued before the heads
                hq0 = 1 if (tails_inline and not CONFIG["ht_defer"]) else 2
                if q >= hq0 and q - hq0 < T:
                    t = q - hq0
                    tcc = t % C
                    for k in range(nchunk):
                        hl = stk[64:116, k * 128:(k + 1) * 128]
                        hr = wg[64:116, HEADCOL:HEADCOL + 1]
                        if not mm_bf16:
                            # f32r rejects N=1 matmuls; run in plain f32
                            hl, hr = hl.bitcast(fp), hr.bitcast(fp)
                        nc.tensor.matmul(pos[:, k * C + tcc:k * C + tcc + 1],
                                         hl, hr, start=True, stop=True)
                    if tcc == C - 1:
                        t0 = t - (C - 1)
                        for k in range(nchunk):
                            # stage PSUM->SBUF on DVE (DMA and Pool can't
                            # read PSUM)
                            nc.vector.tensor_scalar(
                                ost[:, k * C:(k + 1) * C],
                                pos[:, k * C:(k + 1) * C], 0.0, None, add)
                            nc.sync.dma_start(
                                out=out_bt[k * 128:(k + 1) * 128, t0:t0 + C],
                                in_=ost[:, k * C:(k + 1) * C])

                # ---- tails (shared loop unless inlined per group)
                if q <= T and not tails_inline:
                    for g in range(groups):
                        emit_tail(g)
                if q <= T and tails_inline and CONFIG["ht_defer"]:
                    torder = list(range(groups))
                    if hbits is not None:
                        if hbits & 8:
                            torder = torder[::-1]
                    else:
                        if CONFIG["tails_rev"]:
                            torder = torder[::-1]
                        if CONFIG["pingpong"] and q % 2 == 1:
                            torder = torder[::-1]
                    for g in torder:
                        emit_tail(g, "ht")
    nc.compile()
    return nc


def _maybe_uv(nc, g, cols, done, emitted_v, emitted_u, v_eng, u_eng,
              vs, us, ctt, gate_res, add, mult, f_sig):
    if not emitted_v and "i" in done and "g" in done:
        # v = (ti+1)*tg  (= 2*sigma_i*gtilde)
        v_eng.scalar_tensor_tensor(
            vs[g], gate_res(g, "i"), 1.0, gate_res(g, "g"), add, mult)
        emitted_v = True
    if not emitted_u and "f" in done:
        if f_sig:
            # u = sf * ct  (= 2*sigma_f*c)
            u_eng.tensor_tensor(us[g], gate_res(g, "f"), ctt[:, cols], mult)
        else:
            # u = (tf+1)*ct
            u_eng.scalar_tensor_tensor(
                us[g], gate_res(g, "f"), 1.0, ctt[:, cols], add, mult)
        emitted_u = True
    return emitted_v, emitted_u


_NC_CACHE = {}


def _get_nc(T, B, groups=2, use_f32r=True):
    key = (T, B, groups, use_f32r, tuple(sorted(CONFIG.items())))
    if key not in _NC_CACHE:
        _NC_CACHE[key] = build_core_kernel(T, B, groups, use_f32r)
    return _NC_CACHE[key]


def kernel(input, W_ih1, W_hh1, b_ih1, b_hh1, W_ih2, W_hh2, b_ih2, b_hh2,
           W_lin, b_lin, _groups=2, _use_f32r=True):
    from concourse import bass_utils

    input = np.asarray(input, dtype=np.float32)
    B, T = input.shape
    Bc = B // N_CORES
    WG = _build_weights(
        np.asarray(W_ih1, np.float64), np.asarray(W_hh1, np.float64),
        np.asarray(b_ih1, np.float64), np.asarray(b_hh1, np.float64),
        np.asarray(W_ih2, np.float64), np.asarray(W_hh2, np.float64),
        np.asarray(b_ih2, np.float64), np.asarray(b_hh2, np.float64),
        np.asarray(W_lin, np.float64), np.asarray(b_lin, np.float64))
    xT = np.concatenate([np.ones((1, B), np.float32),
                         input.T.astype(np.float32)])
    if CONFIG["mm_bf16"]:
        import ml_dtypes
        WG = WG.astype(ml_dtypes.bfloat16)
        xT = xT.astype(ml_dtypes.bfloat16)
    nc = _get_nc(T, Bc, _groups, _use_f32r)
    in_maps = [
        {"xT": np.ascontiguousarray(xT[:, c * Bc:(c + 1) * Bc]), "WG": WG}
        for c in range(N_CORES)
    ]
    res = bass_utils.run_bass_kernel_spmd(
        nc, in_maps, core_ids=list(range(N_CORES)), trace=False)
    outs = [res.results[c]["out_bt"] for c in range(N_CORES)]
    out = np.concatenate(outs, axis=0)
    return out.astype(np.float32)
